# revision 5
# baseline (speedup 1.0000x reference)
"""Causal multi-head self-attention on 8 Trainium2 NeuronCores.

Problem: x[4, 2048, 1024], 16 heads x d_k=64, torch-Linear-style projections
(weights stored [in, out]), causal softmax attention, output projection.

Sharding (SPMD, one program, per-core data):
  core c -> batch b = c // 2, head-group g = c % 2 (8 heads = 512 model cols).
  QKV column-parallel, output projection row-parallel; the 2-way partial sum
  of the output projection (+ b_o) is done on host at gather time.

Per-core device kernel. All matmul operands are fp16 (cast on host for the
DRAM-resident ones). Accumulation is fp32 in PSUM.

Key structure (v2, tuned from the 306us baseline's trace):
  - Causal mask applied ADDITIVELY on the PE: before the diagonal-block score
    matmul, a [128,128] strictly-lower -512 tile is matmul-accumulated into
    the score PSUM (identity stationary, start=True clears the bank; the
    score matmul rides on top with start=False).  exp then flushes masked
    entries to 0.  This removes the old DVE mask multiply from the per-step
    st -> exp -> pv critical chain.
  - Attention steady state is ACT-bound by ~200ns/step, so PE filler work
    (qk projections for the NEXT head pair, output-projection chains in the
    last pair) is injected between a-steps instead of being lumped.
  - y/denominator are copied out of PSUM to SBUF immediately after the pv
    accumulation stops, so the PSUM bank recycles fast; the reciprocal/
    broadcast/scale chain runs off-band from SBUF.
  - Input DMAs are split across the two HWDGE queues (x on sync, weights +
    biases on scalar) and batched; output stores are [128,1024] fp16 and
    alternate between the queues, interleaved with the final projections.
"""

import sys

sys.path.insert(0, "/opt/trn_rl_repo")

from contextlib import ExitStack

import numpy as np

import concourse.bass as bass  # noqa: F401
import concourse.mybir as mybir
import concourse.tile as tile
from concourse import bacc, bass_utils
from concourse.masks import make_identity, make_lower_triangular

F32 = mybir.dt.float32
F16 = mybir.dt.float16

B, S, D, H, DK = 4, 2048, 1024, 16, 64
NCORE = 8
HPC = 8  # heads per core
DPC = HPC * DK  # model cols per core = 512
NK = D // 128  # k-tiles over the model dim = 8
NST = S // 128  # 128-row S tiles = 16
SCALE = 1.0 / float(np.sqrt(DK))
NEG = -512.0  # additive causal mask value (exp(scale*(s+NEG)) == 0 in fp16)


class Filler:
    """FIFO of PE-work closures injected between attention a-steps."""

    def __init__(self):
        self.q = []
        self.n = 0

    def push(self, fn):
        self.q.append(fn)

    def step(self, every):
        self.n += 1
        if self.q and self.n % every == 0:
            self.q.pop(0)()

    def drain(self):
        while self.q:
            self.q.pop(0)()


def emit(nc, tc, ctx):
    xT = nc.dram_tensor("xT", [D, S], F16, kind="ExternalInput").ap()
    wq = nc.dram_tensor("wq", [D, DPC], F16, kind="ExternalInput").ap()
    wk = nc.dram_tensor("wk", [D, DPC], F16, kind="ExternalInput").ap()
    wv = nc.dram_tensor("wv", [D, DPC], F16, kind="ExternalInput").ap()
    bq = nc.dram_tensor("bq", [DPC], F32, kind="ExternalInput").ap()
    bk = nc.dram_tensor("bk", [DPC], F32, kind="ExternalInput").ap()
    bv = nc.dram_tensor("bv", [DPC], F32, kind="ExternalInput").ap()
    wo = nc.dram_tensor("wo", [DPC, D], F16, kind="ExternalInput").ap()
    out = nc.dram_tensor("out", [S, D], F16, kind="ExternalOutput").ap()

    singles = ctx.enter_context(tc.tile_pool(name="singles", bufs=1))

    # ---- input DMAs: x on the sync queue, weights/biases on scalar ----
    # x resident fp16 [128, S] per k-tile; chunk 0 loaded fine-grained so the
    # first v-projection chain starts as early as possible.
    xt = [singles.tile([128, S], F16, tag=f"xt{k}", name=f"xt{k}") for k in range(NK)]
    for k in range(NK):
        nc.sync.dma_start(out=xt[k][:, 0:512], in_=xT[k * 128:(k + 1) * 128, 0:512])
    for k in range(NK):
        nc.sync.dma_start(out=xt[k][:, 512:S], in_=xT[k * 128:(k + 1) * 128, 512:S])

    # tiny bias loads first: bv_bc and the aps-bank recycling of the first
    # v chains depend on them, and they cost ~0.6us each on the queue
    bv_row = singles.tile([1, DPC], F32, tag="bv_row", name="bv_row")
    nc.scalar.dma_start(out=bv_row, in_=bv.rearrange("(o f) -> o f", o=1))
    # bq/bk packed [128, 4]: column j = bias slice for head pair j
    bq_sb = singles.tile([128, 4], F32, tag="bq_sb", name="bq_sb")
    nc.scalar.dma_start(out=bq_sb, in_=bq.rearrange("(o p) -> p o", p=128))
    bk_sb = singles.tile([128, 4], F32, tag="bk_sb", name="bk_sb")
    nc.scalar.dma_start(out=bk_sb, in_=bk.rearrange("(o p) -> p o", p=128))
    wv_sb = []
    for k in range(NK):
        t = singles.tile([128, DPC], F16, tag=f"wv{k}", name=f"wv{k}")
        nc.scalar.dma_start(out=t, in_=wv[k * 128:(k + 1) * 128, :])
        wv_sb.append(t)
    wq_sb, wk_sb = [], []
    for k in range(NK):
        t = singles.tile([128, DPC], F16, tag=f"wq{k}", name=f"wq{k}")
        nc.scalar.dma_start(out=t, in_=wq[k * 128:(k + 1) * 128, :])
        wq_sb.append(t)
        t = singles.tile([128, DPC], F16, tag=f"wk{k}", name=f"wk{k}")
        nc.scalar.dma_start(out=t, in_=wk[k * 128:(k + 1) * 128, :])
        wk_sb.append(t)
    wo_sb = []
    for k in range(4):
        t = singles.tile([128, D], F16, tag=f"wo{k}", name=f"wo{k}")
        nc.scalar.dma_start(out=t, in_=wo[k * 128:(k + 1) * 128, :])
        wo_sb.append(t)

    # ---- constants / persistent SBUF ----
    ident = singles.tile([128, 128], F16, tag="ident", name="ident")
    make_identity(nc, ident)
    negm = singles.tile([128, 128], F16, tag="negm", name="negm")  # NEG where sk>sq
    make_lower_triangular(nc, negm, val=NEG, diag=False)
    ones8 = singles.tile([128, HPC], F32, tag="ones8", name="ones8")
    nc.vector.memset(ones8, 1.0)
    bv_bc = singles.tile([128, DPC], F32, tag="bv_bc", name="bv_bc")
    nc.gpsimd.partition_broadcast(bv_bc, bv_row)
    # v_store[s]: [128, HPC, 128] fp16 zero-padded pv stationaries: per head
    # slot cols 0-63 = v, col 64 = 1.0 (denominator), cols 65-127 = 0.
    v_store = []
    for s in range(NST):
        t = singles.tile([128, HPC, 128], F16, tag=f"v{s}", name=f"v{s}")
        nc.gpsimd.memset(t[:, :, DK + 1:128], 0.0)
        nc.vector.tensor_copy(out=t[:, :, DK:DK + 1],
                              in_=ones8.rearrange("p (h o) -> p h o", o=1))
        v_store.append(t)
    # kT packed per head pair j: [128, S] (rows 0-63 head 2j, rest 2j+1).
    # qT unpacked per head with the OTHER head's 64 rows zeroed.
    kT = [singles.tile([128, S], F16, tag=f"kT{j}", name=f"kT{j}") for j in range(4)]
    qT = []
    for h in range(HPC):
        t = singles.tile([128, S], F16, tag=f"qTz{h}", name=f"qTz{h}")
        nc.gpsimd.memset(t[(1 - h % 2) * DK:(2 - h % 2) * DK, :], 0.0)
        qT.append(t)
    yT_pack = [singles.tile([128, S], F16, tag=f"yT{j}", name=f"yT{j}") for j in range(4)]

    with tc.tile_pool(name="pa_ps", bufs=2, space="PSUM") as aps, \
         tc.tile_pool(name="pb_p", bufs=5) as p_pool, \
         tc.tile_pool(name="pb_ysb", bufs=3) as ysb_pool, \
         tc.tile_pool(name="pb_div", bufs=3) as div_pool, \
         tc.tile_pool(name="pc_stage", bufs=2) as ostage_pool, \
         tc.tile_pool(name="pb_sps", bufs=2, space="PSUM") as sps, \
         tc.tile_pool(name="pb_yps", bufs=2, space="PSUM") as yps:

        def emit_v_chain(s):
            ps = aps.tile([128, DPC], F32, tag="a", name="aps")
            for k in range(NK):
                nc.tensor.matmul(ps, lhsT=xt[k][:, s * 128:(s + 1) * 128],
                                 rhs=wv_sb[k], start=(k == 0), stop=(k == NK - 1))
            nc.vector.tensor_add(
                out=v_store[s][:, :, 0:DK],
                in0=ps.rearrange("p (h e) -> p h e", e=DK),
                in1=bv_bc.rearrange("p (h e) -> p h e", e=DK))

        def emit_q_chain(j, cc):
            pq = aps.tile([128, 512], F32, tag="a", name="aps")
            for k in range(NK):
                nc.tensor.matmul(pq, lhsT=wq_sb[k][:, j * 128:(j + 1) * 128],
                                 rhs=xt[k][:, cc * 512:(cc + 1) * 512],
                                 start=(k == 0), stop=(k == NK - 1))
            for hh in range(2):
                nc.vector.tensor_scalar_add(
                    out=qT[2 * j + hh][hh * DK:(hh + 1) * DK, cc * 512:(cc + 1) * 512],
                    in0=pq[hh * DK:(hh + 1) * DK, :],
                    scalar1=bq_sb[hh * DK:(hh + 1) * DK, j:j + 1])

        def emit_k_chain(j, cc):
            pk = aps.tile([128, 512], F32, tag="a", name="aps")
            for k in range(NK):
                nc.tensor.matmul(pk, lhsT=wk_sb[k][:, j * 128:(j + 1) * 128],
                                 rhs=xt[k][:, cc * 512:(cc + 1) * 512],
                                 start=(k == 0), stop=(k == NK - 1))
            nc.vector.tensor_scalar_add(out=kT[j][:, cc * 512:(cc + 1) * 512],
                                        in0=pk, scalar1=bk_sb[:, j:j + 1])

        def emit_oproj_stage(s, scalar_copy=False):
            # partial[s-tile] = y_cat @ wo, staged fp16, one [128,1024] store
            stg = ostage_pool.tile([128, D], F16, tag="o", name="ostage")
            for n in range(2):
                ps = aps.tile([128, 512], F32, tag="a", name="aps")
                for kk in range(4):
                    nc.tensor.matmul(ps,
                                     lhsT=yT_pack[kk][:, s * 128:(s + 1) * 128],
                                     rhs=wo_sb[kk][:, n * 512:(n + 1) * 512],
                                     start=(kk == 0), stop=(kk == 3))
                if scalar_copy and n == 1:
                    nc.scalar.copy(out=stg[:, n * 512:(n + 1) * 512], in_=ps)
                else:
                    nc.vector.tensor_copy(out=stg[:, n * 512:(n + 1) * 512], in_=ps)
            # always the sync queue: the scalar queue must stay exp-only
            # while attention is still running
            nc.sync.dma_start(out=out[s * 128:(s + 1) * 128, :], in_=stg)

        def emit_attn(j, cp, inject=None):
            c2 = 2 * cp
            for h in (2 * j, 2 * j + 1):
                po = (h % 2) * DK  # partition offset in the packed k/y tiles
                y_ps = {c: yps.tile([128, 512], F32, tag="y", name="yps")
                        for c in (c2, c2 + 1)}
                for a in range(8 * cp + 8):  # sk tiles: a*128 <= last sq of pair
                    lo = max(0, a * 128 - cp * 1024)  # valid span start in pair
                    has_diag = a * 128 >= cp * 1024
                    st = sps.tile([128, 1024], F32, tag="s", name="sps")
                    pt = p_pool.tile([128, 1024], F16, tag="p", name="pt")
                    diag_pending = has_diag
                    for c in (c2, c2 + 1):
                        x0 = max(lo, (c - c2) * 512)
                        x1 = (c - c2 + 1) * 512
                        if x0 >= x1:
                            continue
                        if diag_pending:
                            # additive causal mask: clears the bank, writes
                            # NEG above the diagonal of block (a, a); the
                            # score matmul is split at lo+128 so each piece
                            # sees a uniform has_written state (accumulate
                            # onto the mask / plain overwrite).
                            nc.tensor.matmul(st[:, lo:lo + 128], lhsT=ident,
                                             rhs=negm, start=True, stop=False)
                            nc.tensor.matmul(
                                st[:, lo:lo + 128],
                                lhsT=kT[j][:, a * 128:(a + 1) * 128],
                                rhs=qT[h][:, cp * 1024 + lo:cp * 1024 + lo + 128],
                                start=False, stop=(lo + 128 == x1))
                            if lo + 128 < x1:
                                nc.tensor.matmul(
                                    st[:, lo + 128:x1],
                                    lhsT=kT[j][:, a * 128:(a + 1) * 128],
                                    rhs=qT[h][:, cp * 1024 + lo + 128:cp * 1024 + x1],
                                    start=False, stop=True)
                            diag_pending = False
                        else:
                            nc.tensor.matmul(
                                st[:, x0:x1],
                                lhsT=kT[j][:, a * 128:(a + 1) * 128],
                                rhs=qT[h][:, cp * 1024 + x0:cp * 1024 + x1],
                                start=True, stop=True)
                    nc.scalar.activation(out=pt[:, lo:1024], in_=st[:, lo:1024],
                                         func=mybir.ActivationFunctionType.Exp,
                                         scale=SCALE)
                    va = v_store[a][:, h, :]  # [128, 128] = [v | 1 | 0...]
                    for c in (c2, c2 + 1):
                        x0 = max(lo, (c - c2) * 512)
                        x1 = (c - c2 + 1) * 512
                        if x0 >= x1:
                            continue
                        nc.tensor.matmul(
                            y_ps[c][:, x0 - (c - c2) * 512:512],
                            lhsT=va, rhs=pt[:, x0:x1],
                            start=(a == 0), stop=(a == 4 * c + 3))
                    if inject is not None:
                        inject()
                # free the y PSUM banks fast, divide off-band from SBUF
                for c in (c2, c2 + 1):
                    drow = div_pool.tile([1, 512], F32, tag="drow", name="drow")
                    nc.vector.tensor_copy(out=drow, in_=y_ps[c][DK:DK + 1, :])
                    y_sb = ysb_pool.tile([DK, 512], F32, tag="ysb", name="ysb")
                    nc.vector.tensor_copy(out=y_sb, in_=y_ps[c][0:DK, :])
                    rrow = div_pool.tile([1, 512], F32, tag="rrow", name="rrow")
                    nc.vector.reciprocal_approx_fast(out=rrow, in_=drow)
                    rbc = div_pool.tile([DK, 512], F32, tag="rbc", name="rbc")
                    nc.gpsimd.partition_broadcast(rbc, rrow)
                    nc.vector.tensor_mul(
                        out=yT_pack[j][po:po + DK, c * 512:(c + 1) * 512],
                        in0=y_sb, in1=rbc)

        # ---- emission schedule ----
        fill = Filler()
        for s in range(8):  # v chunks 0,1
            emit_v_chain(s)
        for cc in (0, 1):  # qk pair 0, first half of S
            emit_q_chain(0, cc)
            emit_k_chain(0, cc)
        for s in range(8, NST):  # v chunks 2,3
            emit_v_chain(s)

        # pair 0: qk0 cc=2,3 injected into cp0; qk1 into cp1
        for cc in (2, 3):
            fill.push(lambda j=0, cc=cc: emit_q_chain(j, cc))
            fill.push(lambda j=0, cc=cc: emit_k_chain(j, cc))
        emit_attn(0, 0, inject=lambda: fill.step(4))
        fill.drain()
        for cc in range(4):
            fill.push(lambda j=1, cc=cc: emit_q_chain(j, cc))
            fill.push(lambda j=1, cc=cc: emit_k_chain(j, cc))
        emit_attn(0, 1, inject=lambda: fill.step(4))
        fill.drain()

        for j, jn in ((1, 2), (2, 3)):
            for cc in range(4):
                fill.push(lambda j=jn, cc=cc: emit_q_chain(j, cc))
                fill.push(lambda j=jn, cc=cc: emit_k_chain(j, cc))
            emit_attn(j, 0, inject=lambda: fill.step(6))
            emit_attn(j, 1, inject=lambda: fill.step(6))
            fill.drain()

        emit_attn(3, 0)
        for s in range(8):
            fill.push(lambda s=s: emit_oproj_stage(s))
        emit_attn(3, 1, inject=lambda: fill.step(4))
        fill.drain()
        for s in range(8, NST):
            emit_oproj_stage(s, scalar_copy=True)


_CACHED_NC = None


def build_program():
    global _CACHED_NC
    if _CACHED_NC is not None:
        return _CACHED_NC
    nc = bacc.Bacc("TRN2", target_bir_lowering=False, debug=False,
                   enable_asserts=False, num_devices=NCORE)
    with tile.TileContext(nc) as tc:
        with ExitStack() as ctx:
            emit(nc, tc, ctx)
    nc.compile()
    _CACHED_NC = nc
    return nc


def shard_inputs(x, w_q, b_q, w_k, b_k, w_v, b_v, w_o):
    f16 = lambda a: np.ascontiguousarray(a, dtype=np.float16)
    in_maps = []
    for c in range(NCORE):
        b, g = divmod(c, 2)
        cols = slice(DPC * g, DPC * (g + 1))
        in_maps.append({
            "xT": f16(x[b].T),
            "wq": f16(w_q[:, cols]),
            "wk": f16(w_k[:, cols]),
            "wv": f16(w_v[:, cols]),
            "bq": np.ascontiguousarray(b_q[cols], dtype=np.float32),
            "bk": np.ascontiguousarray(b_k[cols], dtype=np.float32),
            "bv": np.ascontiguousarray(b_v[cols], dtype=np.float32),
            "wo": f16(w_o[cols, :]),
        })
    return in_maps


def gather_output(results, b_o):
    return np.stack(
        [results[2 * b]["out"].astype(np.float32)
         + results[2 * b + 1]["out"].astype(np.float32) + b_o for b in range(B)]
    ).astype(np.float32)


def kernel(**inputs):
    f = lambda name: np.asarray(inputs[name], dtype=np.float32)
    x, w_q, b_q, w_k, b_k, w_v, b_v, w_o, b_o = (
        f("x"), f("w_q"), f("b_q"), f("w_k"), f("b_k"),
        f("w_v"), f("b_v"), f("w_o"), f("b_o"))
    nc = build_program()
    in_maps = shard_inputs(x, w_q, b_q, w_k, b_k, w_v, b_v, w_o)
    res = bass_utils.run_bass_kernel_spmd(nc, in_maps, core_ids=list(range(NCORE)))
    return gather_output(res.results, b_o)


# revision 10
# speedup vs baseline: 1.0040x; 1.0040x over previous
"""Causal multi-head self-attention on 8 Trainium2 NeuronCores.

Problem: x[4, 2048, 1024], 16 heads x d_k=64, torch-Linear-style projections
(weights stored [in, out]), causal softmax attention, output projection.

Sharding (SPMD, one program, per-core data):
  core c -> batch b = c // 2, head-group g = c % 2 (8 heads = 512 model cols).
  QKV column-parallel, output projection row-parallel; the 2-way partial sum
  of the output projection (+ b_o) is done on host at gather time.

Per-core device kernel. All matmul operands are fp16 (cast on host for the
DRAM-resident ones). Accumulation is fp32 in PSUM.

Key structure (v2, tuned from the 306us baseline's trace):
  - Causal mask applied ADDITIVELY on the PE: before the diagonal-block score
    matmul, a [128,128] strictly-lower -512 tile is matmul-accumulated into
    the score PSUM (identity stationary, start=True clears the bank; the
    score matmul rides on top with start=False).  exp then flushes masked
    entries to 0.  This removes the old DVE mask multiply from the per-step
    st -> exp -> pv critical chain.
  - Attention steady state is ACT-bound by ~200ns/step, so PE filler work
    (qk projections for the NEXT head pair, output-projection chains in the
    last pair) is injected between a-steps instead of being lumped.
  - y/denominator are copied out of PSUM to SBUF immediately after the pv
    accumulation stops, so the PSUM bank recycles fast; the reciprocal/
    broadcast/scale chain runs off-band from SBUF.
  - Input DMAs are split across the two HWDGE queues (x on sync, weights +
    biases on scalar) and batched; output stores are [128,1024] fp16 and
    alternate between the queues, interleaved with the final projections.
"""

import sys

sys.path.insert(0, "/opt/trn_rl_repo")

from contextlib import ExitStack

import numpy as np

import concourse.bass as bass  # noqa: F401
import concourse.mybir as mybir
import concourse.tile as tile
from concourse import bacc, bass_utils
from concourse.masks import make_identity, make_lower_triangular

F32 = mybir.dt.float32
F16 = mybir.dt.float16

B, S, D, H, DK = 4, 2048, 1024, 16, 64
NCORE = 8
HPC = 8  # heads per core
DPC = HPC * DK  # model cols per core = 512
NK = D // 128  # k-tiles over the model dim = 8
NST = S // 128  # 128-row S tiles = 16
SCALE = 1.0 / float(np.sqrt(DK))
NEG = -512.0  # additive causal mask value (exp(scale*(s+NEG)) == 0 in fp16)


class Filler:
    """FIFO of PE-work closures injected between attention a-steps."""

    def __init__(self):
        self.q = []
        self.n = 0

    def push(self, fn):
        self.q.append(fn)

    def step(self, every):
        self.n += 1
        if self.q and self.n % every == 0:
            self.q.pop(0)()

    def drain(self):
        while self.q:
            self.q.pop(0)()


def emit(nc, tc, ctx):
    xT = nc.dram_tensor("xT", [D, S], F16, kind="ExternalInput").ap()
    wq = nc.dram_tensor("wq", [D, DPC], F16, kind="ExternalInput").ap()
    wk = nc.dram_tensor("wk", [D, DPC], F16, kind="ExternalInput").ap()
    wv = nc.dram_tensor("wv", [D, DPC], F16, kind="ExternalInput").ap()
    bq = nc.dram_tensor("bq", [DPC], F32, kind="ExternalInput").ap()
    bk = nc.dram_tensor("bk", [DPC], F32, kind="ExternalInput").ap()
    bv = nc.dram_tensor("bv", [DPC], F32, kind="ExternalInput").ap()
    wo = nc.dram_tensor("wo", [DPC, D], F16, kind="ExternalInput").ap()
    out = nc.dram_tensor("out", [S, D], F16, kind="ExternalOutput").ap()

    singles = ctx.enter_context(tc.tile_pool(name="singles", bufs=1))

    # ---- input DMAs: x on the sync queue, weights/biases on scalar ----
    # x resident fp16 [128, S] per k-tile; chunk 0 loaded fine-grained so the
    # first v-projection chain starts as early as possible.
    xt = [singles.tile([128, S], F16, tag=f"xt{k}", name=f"xt{k}") for k in range(NK)]
    for k in range(NK):
        nc.sync.dma_start(out=xt[k][:, 0:512], in_=xT[k * 128:(k + 1) * 128, 0:512])
    for k in range(NK):
        nc.sync.dma_start(out=xt[k][:, 512:S], in_=xT[k * 128:(k + 1) * 128, 512:S])
    wk_sb = []
    for k in range(NK):  # wk on the sync queue balances the two HWDGE queues
        t = singles.tile([128, DPC], F16, tag=f"wk{k}", name=f"wk{k}")
        nc.sync.dma_start(out=t, in_=wk[k * 128:(k + 1) * 128, :])
        wk_sb.append(t)

    # tiny bias loads first: bv_bc and the aps-bank recycling of the first
    # v chains depend on them, and they cost ~0.6us each on the queue
    bv_row = singles.tile([1, DPC], F32, tag="bv_row", name="bv_row")
    nc.scalar.dma_start(out=bv_row, in_=bv.rearrange("(o f) -> o f", o=1))
    # bq/bk packed [128, 4]: column j = bias slice for head pair j
    bq_sb = singles.tile([128, 4], F32, tag="bq_sb", name="bq_sb")
    nc.scalar.dma_start(out=bq_sb, in_=bq.rearrange("(o p) -> p o", p=128))
    bk_sb = singles.tile([128, 4], F32, tag="bk_sb", name="bk_sb")
    nc.scalar.dma_start(out=bk_sb, in_=bk.rearrange("(o p) -> p o", p=128))
    wv_sb = []
    for k in range(NK):
        t = singles.tile([128, DPC], F16, tag=f"wv{k}", name=f"wv{k}")
        nc.scalar.dma_start(out=t, in_=wv[k * 128:(k + 1) * 128, :])
        wv_sb.append(t)
    wq_sb = []
    for k in range(NK):
        t = singles.tile([128, DPC], F16, tag=f"wq{k}", name=f"wq{k}")
        nc.scalar.dma_start(out=t, in_=wq[k * 128:(k + 1) * 128, :])
        wq_sb.append(t)
    wo_sb = []
    for k in range(4):
        t = singles.tile([128, D], F16, tag=f"wo{k}", name=f"wo{k}")
        nc.scalar.dma_start(out=t, in_=wo[k * 128:(k + 1) * 128, :])
        wo_sb.append(t)

    # ---- constants / persistent SBUF ----
    ident = singles.tile([128, 128], F16, tag="ident", name="ident")
    make_identity(nc, ident)
    negm = singles.tile([128, 128], F16, tag="negm", name="negm")  # NEG where sk>sq
    make_lower_triangular(nc, negm, val=NEG, diag=False)
    ones8 = singles.tile([128, HPC], F32, tag="ones8", name="ones8")
    nc.vector.memset(ones8, 1.0)
    bv_bc = singles.tile([128, DPC], F32, tag="bv_bc", name="bv_bc")
    nc.gpsimd.partition_broadcast(bv_bc, bv_row)
    # v_store[s]: [128, HPC, 128] fp16 zero-padded pv stationaries: per head
    # slot cols 0-63 = v, col 64 = 1.0 (denominator), cols 65-127 = 0.
    v_store = []
    for s in range(NST):
        t = singles.tile([128, HPC, 128], F16, tag=f"v{s}", name=f"v{s}")
        nc.gpsimd.memset(t[:, :, DK + 1:128], 0.0)
        nc.vector.tensor_copy(out=t[:, :, DK:DK + 1],
                              in_=ones8.rearrange("p (h o) -> p h o", o=1))
        v_store.append(t)
    # kT packed per head pair j: [128, S] (rows 0-63 head 2j, rest 2j+1).
    # qT unpacked per head with the OTHER head's 64 rows zeroed.
    kT = [singles.tile([128, S], F16, tag=f"kT{j}", name=f"kT{j}") for j in range(4)]
    qT = []
    for h in range(HPC):
        t = singles.tile([128, S], F16, tag=f"qTz{h}", name=f"qTz{h}")
        nc.gpsimd.memset(t[(1 - h % 2) * DK:(2 - h % 2) * DK, :], 0.0)
        qT.append(t)
    yT_pack = [singles.tile([128, S], F16, tag=f"yT{j}", name=f"yT{j}") for j in range(4)]

    with tc.tile_pool(name="pa_ps", bufs=2, space="PSUM") as aps, \
         tc.tile_pool(name="pb_p", bufs=5) as p_pool, \
         tc.tile_pool(name="pb_ysb", bufs=3) as ysb_pool, \
         tc.tile_pool(name="pb_div", bufs=3) as div_pool, \
         tc.tile_pool(name="pc_stage", bufs=2) as ostage_pool, \
         tc.tile_pool(name="pb_sps", bufs=2, space="PSUM") as sps, \
         tc.tile_pool(name="pb_yps", bufs=2, space="PSUM") as yps:

        def emit_v_chain(s):
            ps = aps.tile([128, DPC], F32, tag="a", name="aps")
            for k in range(NK):
                nc.tensor.matmul(ps, lhsT=xt[k][:, s * 128:(s + 1) * 128],
                                 rhs=wv_sb[k], start=(k == 0), stop=(k == NK - 1))
            nc.vector.tensor_add(
                out=v_store[s][:, :, 0:DK],
                in0=ps.rearrange("p (h e) -> p h e", e=DK),
                in1=bv_bc.rearrange("p (h e) -> p h e", e=DK))

        def emit_q_chain(j, cc):
            pq = aps.tile([128, 512], F32, tag="a", name="aps")
            for k in range(NK):
                nc.tensor.matmul(pq, lhsT=wq_sb[k][:, j * 128:(j + 1) * 128],
                                 rhs=xt[k][:, cc * 512:(cc + 1) * 512],
                                 start=(k == 0), stop=(k == NK - 1))
            for hh in range(2):
                nc.vector.tensor_scalar_add(
                    out=qT[2 * j + hh][hh * DK:(hh + 1) * DK, cc * 512:(cc + 1) * 512],
                    in0=pq[hh * DK:(hh + 1) * DK, :],
                    scalar1=bq_sb[hh * DK:(hh + 1) * DK, j:j + 1])

        def emit_k_chain(j, cc):
            pk = aps.tile([128, 512], F32, tag="a", name="aps")
            for k in range(NK):
                nc.tensor.matmul(pk, lhsT=wk_sb[k][:, j * 128:(j + 1) * 128],
                                 rhs=xt[k][:, cc * 512:(cc + 1) * 512],
                                 start=(k == 0), stop=(k == NK - 1))
            nc.vector.tensor_scalar_add(out=kT[j][:, cc * 512:(cc + 1) * 512],
                                        in0=pk, scalar1=bk_sb[:, j:j + 1])

        def emit_oproj_stage(s, tail=False):
            # partial[s-tile] = y_cat @ wo, staged fp16, one [128,1024] store.
            # While attention still runs the scalar queue must stay exp-only,
            # so interleaved stages copy on vector and store on sync; tail
            # stages spread across both engines/queues.
            stg = ostage_pool.tile([128, D], F16, tag="o", name="ostage")
            for n in range(2):
                ps = aps.tile([128, 512], F32, tag="a", name="aps")
                for kk in range(4):
                    nc.tensor.matmul(ps,
                                     lhsT=yT_pack[kk][:, s * 128:(s + 1) * 128],
                                     rhs=wo_sb[kk][:, n * 512:(n + 1) * 512],
                                     start=(kk == 0), stop=(kk == 3))
                if tail and n == 1:
                    nc.scalar.copy(out=stg[:, n * 512:(n + 1) * 512], in_=ps)
                else:
                    nc.vector.tensor_copy(out=stg[:, n * 512:(n + 1) * 512], in_=ps)
            eng = nc.scalar if (tail and s % 2 == 1) else nc.sync
            eng.dma_start(out=out[s * 128:(s + 1) * 128, :], in_=stg)

        def emit_attn(j, cp, inject=None):
            c2 = 2 * cp
            nsteps = 8 * cp + 8  # sk tiles: a*128 <= last sq of pair
            for h in (2 * j, 2 * j + 1):
                po = (h % 2) * DK  # partition offset in the packed k/y tiles
                y_ps = {c: yps.tile([128, 512], F32, tag="y", name="yps")
                        for c in (c2, c2 + 1)}
                prepped = {}

                def prep_step(a):
                    # Allocate st/pt and pre-write the additive causal mask
                    # (NEG above the diagonal of block (a,a); start=True
                    # clears the bank so the later score matmuls overwrite /
                    # accumulate per has_written).  Emitted one step early so
                    # the identity LDWEIGHTS hides under long matmuls.
                    lo = max(0, a * 128 - cp * 1024)
                    st = sps.tile([128, 1024], F32, tag="s", name="sps")
                    pt = p_pool.tile([128, 1024], F16, tag="p", name="pt")
                    if a * 128 >= cp * 1024:  # has diagonal
                        nc.tensor.matmul(st[:, lo:lo + 128], lhsT=ident,
                                         rhs=negm, start=True, stop=False)
                    prepped[a] = (st, pt)

                prep_step(0)
                for a in range(nsteps):
                    lo = max(0, a * 128 - cp * 1024)  # valid span start
                    has_diag = a * 128 >= cp * 1024
                    st, pt = prepped.pop(a)
                    diag_pending = has_diag
                    for c in (c2, c2 + 1):
                        x0 = max(lo, (c - c2) * 512)
                        x1 = (c - c2 + 1) * 512
                        if x0 >= x1:
                            continue
                        if diag_pending:
                            # score split at lo+128: accumulate onto the mask
                            # in [lo, lo+128), plain overwrite beyond it
                            nc.tensor.matmul(
                                st[:, lo:lo + 128],
                                lhsT=kT[j][:, a * 128:(a + 1) * 128],
                                rhs=qT[h][:, cp * 1024 + lo:cp * 1024 + lo + 128],
                                start=False, stop=(lo + 128 == x1))
                            if lo + 128 < x1:
                                nc.tensor.matmul(
                                    st[:, lo + 128:x1],
                                    lhsT=kT[j][:, a * 128:(a + 1) * 128],
                                    rhs=qT[h][:, cp * 1024 + lo + 128:cp * 1024 + x1],
                                    start=False, stop=True)
                            diag_pending = False
                        else:
                            nc.tensor.matmul(
                                st[:, x0:x1],
                                lhsT=kT[j][:, a * 128:(a + 1) * 128],
                                rhs=qT[h][:, cp * 1024 + x0:cp * 1024 + x1],
                                start=True, stop=True)
                    nc.scalar.activation(out=pt[:, lo:1024], in_=st[:, lo:1024],
                                         func=mybir.ActivationFunctionType.Exp,
                                         scale=SCALE)
                    if a + 1 < nsteps:
                        prep_step(a + 1)
                    va = v_store[a][:, h, :]  # [128, 128] = [v | 1 | 0...]
                    for c in (c2, c2 + 1):
                        x0 = max(lo, (c - c2) * 512)
                        x1 = (c - c2 + 1) * 512
                        if x0 >= x1:
                            continue
                        nc.tensor.matmul(
                            y_ps[c][:, x0 - (c - c2) * 512:512],
                            lhsT=va, rhs=pt[:, x0:x1],
                            start=(a == 0), stop=(a == 4 * c + 3))
                    if inject is not None:
                        inject()
                # free the y PSUM banks fast, divide off-band from SBUF
                for c in (c2, c2 + 1):
                    drow = div_pool.tile([1, 512], F32, tag="drow", name="drow")
                    nc.vector.tensor_copy(out=drow, in_=y_ps[c][DK:DK + 1, :])
                    y_sb = ysb_pool.tile([DK, 512], F32, tag="ysb", name="ysb")
                    nc.vector.tensor_copy(out=y_sb, in_=y_ps[c][0:DK, :])
                    rrow = div_pool.tile([1, 512], F32, tag="rrow", name="rrow")
                    nc.vector.reciprocal_approx_fast(out=rrow, in_=drow)
                    rbc = div_pool.tile([DK, 512], F32, tag="rbc", name="rbc")
                    nc.gpsimd.partition_broadcast(rbc, rrow)
                    nc.vector.tensor_mul(
                        out=yT_pack[j][po:po + DK, c * 512:(c + 1) * 512],
                        in0=y_sb, in1=rbc)

        # ---- emission schedule ----
        fill = Filler()
        for s in range(8):  # v chunks 0,1
            emit_v_chain(s)
        for cc in (0, 1):  # qk pair 0, first half of S
            emit_q_chain(0, cc)
            emit_k_chain(0, cc)
        for s in range(8, NST):  # v chunks 2,3
            emit_v_chain(s)

        # pair 0: qk0 cc=2,3 injected into cp0; qk1 into cp1
        for cc in (2, 3):
            fill.push(lambda j=0, cc=cc: emit_q_chain(j, cc))
            fill.push(lambda j=0, cc=cc: emit_k_chain(j, cc))
        emit_attn(0, 0, inject=lambda: fill.step(4))
        fill.drain()
        for cc in range(4):
            fill.push(lambda j=1, cc=cc: emit_q_chain(j, cc))
            fill.push(lambda j=1, cc=cc: emit_k_chain(j, cc))
        emit_attn(0, 1, inject=lambda: fill.step(4))
        fill.drain()

        for j, jn in ((1, 2), (2, 3)):
            for cc in range(4):
                fill.push(lambda j=jn, cc=cc: emit_q_chain(j, cc))
                fill.push(lambda j=jn, cc=cc: emit_k_chain(j, cc))
            emit_attn(j, 0, inject=lambda: fill.step(6))
            emit_attn(j, 1, inject=lambda: fill.step(6))
            fill.drain()

        emit_attn(3, 0)
        for s in range(8):
            fill.push(lambda s=s: emit_oproj_stage(s))
        emit_attn(3, 1, inject=lambda: fill.step(4))
        fill.drain()
        for s in range(8, NST):
            emit_oproj_stage(s, tail=True)


_CACHED_NC = None


def build_program():
    global _CACHED_NC
    if _CACHED_NC is not None:
        return _CACHED_NC
    nc = bacc.Bacc("TRN2", target_bir_lowering=False, debug=False,
                   enable_asserts=False, num_devices=NCORE)
    with tile.TileContext(nc) as tc:
        with ExitStack() as ctx:
            emit(nc, tc, ctx)
    nc.compile()
    _CACHED_NC = nc
    return nc


def shard_inputs(x, w_q, b_q, w_k, b_k, w_v, b_v, w_o):
    f16 = lambda a: np.ascontiguousarray(a, dtype=np.float16)
    in_maps = []
    for c in range(NCORE):
        b, g = divmod(c, 2)
        cols = slice(DPC * g, DPC * (g + 1))
        in_maps.append({
            "xT": f16(x[b].T),
            "wq": f16(w_q[:, cols]),
            "wk": f16(w_k[:, cols]),
            "wv": f16(w_v[:, cols]),
            "bq": np.ascontiguousarray(b_q[cols], dtype=np.float32),
            "bk": np.ascontiguousarray(b_k[cols], dtype=np.float32),
            "bv": np.ascontiguousarray(b_v[cols], dtype=np.float32),
            "wo": f16(w_o[cols, :]),
        })
    return in_maps


def gather_output(results, b_o):
    return np.stack(
        [results[2 * b]["out"].astype(np.float32)
         + results[2 * b + 1]["out"].astype(np.float32) + b_o for b in range(B)]
    ).astype(np.float32)


def kernel(**inputs):
    f = lambda name: np.asarray(inputs[name], dtype=np.float32)
    x, w_q, b_q, w_k, b_k, w_v, b_v, w_o, b_o = (
        f("x"), f("w_q"), f("b_q"), f("w_k"), f("b_k"),
        f("w_v"), f("b_v"), f("w_o"), f("b_o"))
    nc = build_program()
    in_maps = shard_inputs(x, w_q, b_q, w_k, b_k, w_v, b_v, w_o)
    res = bass_utils.run_bass_kernel_spmd(nc, in_maps, core_ids=list(range(NCORE)))
    return gather_output(res.results, b_o)
